# revision 1
# baseline (speedup 1.0000x reference)
# Transformer-XL decoder (4 layers) on 8 trn2 NeuronCores.
# Sharding: core = (batch b, head-group g); b = core//2, g = core%2.
# Each core: 4 heads of attention for its batch + half of d_inner for the FFN.
# Pair cores (2b, 2b+1) AllReduce the o-projection and FF2 partial sums.
#
# Device layouts (per core):
#   activations feature-major: hT [128, 4*1024]  (dm = a*128+p at col a*1024+t)
#   k/rk/q'/q'' per head-pair, rows hh*64+d (hh = head-in-pair, K=64 row-
#     packed matmuls with base partitions {0, 64})
#   v token-major: v_sb [128, 16*272]; window jw at col jw*272, head h at
#     +h*68 (64 cols + a ones col at +64 for the softmax denominator)
#   scores S[i-part, j-free]; BDpre[i-part, r-free], r = j - i + 1023.
#     The valid band for q-tile it starts at r_lo = 896-128*it, so the skew
#     source column is q = j + 127 - p (it-independent): one skewed
#     SBUF->SBUF DMA with accum_op=add adds the rel-shifted BD onto the
#     evicted AC; 128 pad cols of -1e4 land on causally-masked j.
#   exp in place (ACT), xbar dma_start_transpose per 128-col block ->
#   P^T [j-part, i-free], AV with lhsT=[v|1] (M=65, row 64 = denominator).

import os
import numpy as np
import ml_dtypes

NL, NH, DM, DH, DI = 4, 8, 512, 64, 2048
QLEN, MLEN, B = 1024, 1024, 4
KLEN = QLEN + MLEN
NCORES = 8
HPC = NH // 2        # heads per core
PAIRS = HPC // 2     # head pairs per core
DIH = DI // 2        # d_inner slice per core
SC = 1.0 / np.sqrt(DH)
NEG = -10000.0
NBLK = DM // 128
NTOK = QLEN
VPITCH = 68
WBD = 2304           # bd_sb width: max band 2048 + 128 pad, 128-aligned

bf16 = ml_dtypes.bfloat16

_COMPILED = None


def _v(it):
    return 128 * (9 + it)        # valid j width for q-tile it


def _njc(it):
    return (_v(it) + 511) // 512


def _r0a(it):
    return 512 * ((896 - 128 * it) // 512)


def build_program(n_layers=NL):
    import concourse.bass as bass
    from concourse import bacc
    import concourse.tile as tile
    import concourse.mybir as mybir
    from concourse.ap import AP
    from concourse import library_config

    f32 = mybir.dt.float32
    b16 = mybir.dt.bfloat16
    AF = mybir.ActivationFunctionType
    GROUPS = [[0, 1], [2, 3], [4, 5], [6, 7]]

    nc = bacc.Bacc("TRN2", num_devices=NCORES, debug=False)

    h0T = nc.dram_tensor("h0T", [DM, NTOK], f32, kind="ExternalInput").ap()
    memsT = nc.dram_tensor("memsT", [NL, DM, MLEN], b16, kind="ExternalInput").ap()
    posT = nc.dram_tensor("posT", [DM, KLEN], b16, kind="ExternalInput").ap()
    qkvwT = nc.dram_tensor("qkvwT", [NL, DM, 768], b16, kind="ExternalInput").ap()
    rwT = nc.dram_tensor("rwT", [NL, DM, 256], b16, kind="ExternalInput").ap()
    owT = nc.dram_tensor("owT", [NL, 256, DM], b16, kind="ExternalInput").ap()
    ff1T = nc.dram_tensor("ff1T", [NL, DM, DIH], b16, kind="ExternalInput").ap()
    ff2T = nc.dram_tensor("ff2T", [NL, DIH, DM], b16, kind="ExternalInput").ap()
    bias = nc.dram_tensor("bias", [128, 38 * NL], f32, kind="ExternalInput").ap()
    yT = nc.dram_tensor("yT", [DM, NTOK], f32, kind="ExternalOutput").ap()

    # bias column map (keep in sync with _pack_core_inputs):
    # base = 38*l: +0,1 qb_w(pair) | +2,3 qb_r | +4,5 kb | +6..9 ob |
    # +10..17 ff1b | +18..21 ff2b/2 | +22..25 ln1_s | +26..29 ln1_b |
    # +30..33 ln2_s | +34..37 ln2_b

    with tile.TileContext(nc) as tc:
      with tc.tile_pool(name="persist", bufs=1) as pp_, \
           tc.tile_pool(name="dramp", bufs=2, space="DRAM") as dram:
        hT = pp_.tile([128, NBLK * NTOK], f32)
        hTb = pp_.tile([128, NBLK * NTOK], b16)
        posTs = pp_.tile([128, NBLK * KLEN], b16)
        bias_sb = pp_.tile([128, 38 * NL], f32)
        lt = pp_.tile([128, NBLK * NTOK], b16)
        ones_c = pp_.tile([128, 1], b16)
        eps_c = pp_.tile([128, 1], f32)

        def load_blocked(dst_ap, src2d, nb):
            nc.sync.dma_start(dst_ap.rearrange("p (a t) -> p a t", a=nb),
                              src2d.rearrange("(a p) t -> p a t", p=128))

        def bcol(l, off):
            return bias_sb[:, 38 * l + off: 38 * l + off + 1]

        if not os.environ.get("NO_LIB"):
            nc.gpsimd.load_library(library_config.mlp)
        nc.gpsimd.memset(ones_c[:], 1.0)
        nc.gpsimd.memset(eps_c[:], 1e-5)
        nc.sync.dma_start(bias_sb[:], bias)
        load_blocked(hT[:], h0T, NBLK)
        load_blocked(posTs[:], posT, NBLK)
        nc.vector.tensor_copy(hTb[:], hT[:])

        def layer_norm(l, boff):
            """LN over dm (partition blocks) of hT, in place; refresh hTb."""
            with tc.tile_pool(name="lns", bufs=1) as lp, \
                 tc.tile_pool(name="pl", bufs=2, space="PSUM") as pl:
                nc.vector.tensor_copy(lt[:], hT[:])
                hsq = lp.tile([128, NBLK * 512], b16, tag="hsq")
                mu = lp.tile([1, NTOK], f32, tag="mu")
                ex2 = lp.tile([1, NTOK], f32, tag="ex2")
                tmp = lp.tile([1, NTOK], f32, tag="tmp")
                invc = lp.tile([1, 2 * NTOK], f32, tag="invc")
                invb = lp.tile([128, 2 * NTOK], f32, tag="invb")
                for t2 in range(2):
                    sx = pl.tile([128, 512], f32, tag="sx")
                    sq = pl.tile([128, 512], f32, tag="sq")
                    for a in range(NBLK):
                        seg = lt[:, a * NTOK + t2 * 512: a * NTOK + t2 * 512 + 512]
                        nc.tensor.matmul(sx[0:1, :], ones_c[:], seg,
                                         start=(a == 0), stop=(a == NBLK - 1))
                        hs = hsq[:, a * 512:(a + 1) * 512]
                        nc.scalar.activation(hs, seg, AF.Square)
                        nc.tensor.matmul(sq[0:1, :], ones_c[:], hs,
                                         start=(a == 0), stop=(a == NBLK - 1))
                    ts = slice(t2 * 512, t2 * 512 + 512)
                    nc.scalar.mul(mu[:, ts], sx[0:1, :], 1.0 / DM)
                    nc.scalar.mul(ex2[:, ts], sq[0:1, :], 1.0 / DM)
                nc.vector.tensor_mul(tmp[:], mu[:], mu[:])      # mu^2
                nc.vector.tensor_sub(ex2[:], ex2[:], tmp[:])    # var
                nc.scalar.activation(tmp[:], ex2[:], AF.Sqrt, bias=eps_c[0:1, :])
                inv = invc[:, 0:NTOK]
                cneg = invc[:, NTOK:2 * NTOK]
                nc.vector.reciprocal(inv, tmp[:])
                nc.vector.tensor_mul(cneg, mu[:], inv)
                nc.scalar.mul(cneg, cneg, -1.0)
                if os.environ.get("NO_BCAST_LN"):
                    nc.gpsimd.memset(invb[:], 1.0)
                else:
                    nc.gpsimd.partition_broadcast(invb[:], invc[:])
                for a in range(NBLK):
                    asl = slice(a * NTOK, (a + 1) * NTOK)
                    nc.vector.tensor_mul(lt[:, asl], hT[:, asl],
                                         invb[:, 0:NTOK])
                    nc.vector.tensor_add(lt[:, asl], lt[:, asl],
                                         invb[:, NTOK:2 * NTOK])
                    nc.scalar.activation(hT[:, asl], lt[:, asl], AF.Identity,
                                         bias=bcol(l, boff + 4 + a),
                                         scale=bcol(l, boff + a))
                nc.vector.tensor_copy(hTb[:], hT[:])

        for l in range(n_layers):
          with tc.tile_pool(name="wa", bufs=1) as wa, \
               tc.tile_pool(name="attn_in", bufs=1) as ai:
            memsL = wa.tile([128, NBLK * MLEN], b16)
            load_blocked(memsL[:], memsT[l], NBLK)
            qkvw = wa.tile([128, NBLK * 768], b16)
            load_blocked(qkvw[:], qkvwT[l], NBLK)
            rww = wa.tile([128, NBLK * 256], b16)
            load_blocked(rww[:], rwT[l], NBLK)

            k_sb = [ai.tile([128, KLEN], b16, tag=f"k_sb{i_}", name=f"k_sb{i_}") for i_ in range(PAIRS)]
            rk_sb = [ai.tile([128, KLEN], b16, tag=f"rk_sb{i_}", name=f"rk_sb{i_}") for i_ in range(PAIRS)]
            qw_sb = [ai.tile([128, NTOK], b16, tag=f"qw_sb{i_}", name=f"qw_sb{i_}") for i_ in range(PAIRS)]
            qr_sb = [ai.tile([128, NTOK], b16, tag=f"qr_sb{i_}", name=f"qr_sb{i_}") for i_ in range(PAIRS)]
            v_sb = ai.tile([128, 16 * 272], b16, tag="v_sb")
            av_sb = ai.tile([128, 2 * NTOK], b16, tag="av_sb")

            def cat_rhs(a, j0, n):
                if j0 < MLEN:
                    return memsL[:, a * MLEN + j0: a * MLEN + j0 + n]
                t0 = j0 - MLEN
                return hTb[:, a * NTOK + t0: a * NTOK + t0 + n]

            # ---------------- phase A: qkv + rk ----------------
            with tc.tile_pool(name="pa", bufs=4, space="PSUM") as pa, \
                 tc.tile_pool(name="pv", bufs=2, space="PSUM") as pv:
                for ppi in range(PAIRS):
                    for jc in range(4):
                        ps = pa.tile([128, 512], f32, tag="pa")
                        for a in range(NBLK):
                            lhs = qkvw[:, a * 768 + 256 + ppi * 128:
                                       a * 768 + 256 + ppi * 128 + 128]
                            nc.tensor.matmul(ps[:], lhs,
                                             cat_rhs(a, jc * 512, 512),
                                             start=(a == 0), stop=(a == NBLK - 1))
                        nc.scalar.activation(
                            k_sb[ppi][:, jc * 512:(jc + 1) * 512], ps[:],
                            AF.Identity, bias=bcol(l, 4 + ppi))
                for ppi in range(PAIRS):
                    for jc in range(2):
                        ps = pa.tile([128, 512], f32, tag="pa")
                        for a in range(NBLK):
                            lhs = qkvw[:, a * 768 + ppi * 128:
                                       a * 768 + ppi * 128 + 128]
                            nc.tensor.matmul(
                                ps[:], lhs,
                                hTb[:, a * NTOK + jc * 512: a * NTOK + jc * 512 + 512],
                                start=(a == 0), stop=(a == NBLK - 1))
                        nc.scalar.activation(
                            qw_sb[ppi][:, jc * 512:(jc + 1) * 512], ps[:],
                            AF.Identity, bias=bcol(l, 0 + ppi))
                        nc.scalar.activation(
                            qr_sb[ppi][:, jc * 512:(jc + 1) * 512], ps[:],
                            AF.Identity, bias=bcol(l, 2 + ppi))
                for ppi in range(PAIRS):
                    for jc in range(4):
                        ps = pa.tile([128, 512], f32, tag="pa")
                        for a in range(NBLK):
                            lhs = rww[:, a * 256 + ppi * 128:
                                      a * 256 + ppi * 128 + 128]
                            nc.tensor.matmul(
                                ps[:], lhs,
                                posTs[:, a * KLEN + jc * 512: a * KLEN + jc * 512 + 512],
                                start=(a == 0), stop=(a == NBLK - 1))
                        nc.vector.tensor_copy(
                            rk_sb[ppi][:, jc * 512:(jc + 1) * 512], ps[:])
                nc.gpsimd.memset(v_sb[:], 1.0)
                for jw in range(16):
                    ps = pv.tile([128, 256], f32, tag="pv")
                    for a in range(NBLK):
                        lhs = cat_rhs(a, jw * 128, 128)
                        nc.tensor.matmul(ps[:], lhs,
                                         qkvw[:, a * 768 + 512: a * 768 + 768],
                                         start=(a == 0), stop=(a == NBLK - 1))
                    dst = AP(v_sb[:].tensor, v_sb[:].offset + jw * 272,
                             [[16 * 272, 128], [VPITCH, HPC], [1, 64]])
                    nc.vector.tensor_copy(
                        dst, ps[:].rearrange("p (h d) -> p h d", d=64))

            # ---------------- phase B: attention ----------------
            with tc.tile_pool(name="bd", bufs=2) as bdp, \
                 tc.tile_pool(name="p_sb", bufs=9) as psb, \
                 tc.tile_pool(name="pT", bufs=3) as ptp, \
                 tc.tile_pool(name="avs", bufs=2) as avsp, \
                 tc.tile_pool(name="sc", bufs=3, space="PSUM") as scp, \
                 tc.tile_pool(name="av", bufs=2, space="PSUM") as avp:
                for ppi in range(PAIRS):
                    p_tiles = {}
                    for ig in range(2):
                        for itg in range(4):
                            it = ig * 4 + itg
                            W = _v(it)
                            r0a = _r0a(it)
                            rlo = 896 - 128 * it
                            i0 = it * 128
                            bw = 1152 + 128 * it
                            for hh in range(2):
                                hs = slice(hh * 64, hh * 64 + 64)
                                # BDpre [i, r], r in [r0a, 2048)
                                nrc = (KLEN - r0a) // 512
                                nbp = (nrc + 1) // 2
                                bps = [scp.tile([128, 1024], f32, tag="sc", name=f"bps{i_}")
                                       for i_ in range(nbp)]
                                for rc in range(nrc):
                                    nc.tensor.matmul(
                                        bps[rc // 2][:, (rc % 2) * 512:(rc % 2) * 512 + 512],
                                        qr_sb[ppi][hs, i0:i0 + 128],
                                        rk_sb[ppi][hs, r0a + rc * 512: r0a + (rc + 1) * 512],
                                        start=True, stop=True)
                                bd = bdp.tile([128, WBD], b16, tag="bd")
                                off0 = rlo - r0a
                                w0 = 1024 - off0
                                nc.vector.tensor_copy(bd[:, 0:w0],
                                                      bps[0][:, off0:1024])
                                done = w0
                                for ti in range(1, nbp):
                                    seg = min(1024, (KLEN - r0a) - ti * 1024)
                                    nc.vector.tensor_copy(
                                        bd[:, done:done + seg], bps[ti][:, 0:seg])
                                    done += seg
                                assert done == bw
                                nc.gpsimd.memset(bd[:, bw:bw + 128], NEG)
                                # AC [i, j], j in [0, W)
                                nap = (W + 1023) // 1024
                                aps = [scp.tile([128, 1024], f32, tag="sc", name=f"aps{i_}")
                                       for i_ in range(nap)]
                                for jc in range(_njc(it)):
                                    n = min(512, W - jc * 512)
                                    nc.tensor.matmul(
                                        aps[jc // 2][:, (jc % 2) * 512:(jc % 2) * 512 + n],
                                        qw_sb[ppi][hs, i0:i0 + 128],
                                        k_sb[ppi][hs, jc * 512: jc * 512 + n],
                                        start=True, stop=True)
                                pS = psb.tile([128, 2048], b16, tag="p_sb")
                                done = 0
                                for ti in range(nap):
                                    seg = min(1024, W - ti * 1024)
                                    nc.vector.tensor_copy(
                                        pS[:, done:done + seg], aps[ti][:, 0:seg])
                                    done += seg
                                if not os.environ.get("NO_SKEW"):
                                    src = AP(bd[:].tensor, bd[:].offset + 127,
                                             [[WBD - 1, 128], [1, W]])
                                    nc.gpsimd.dma_start(pS[:, 0:W], src,
                                                        accum_op=mybir.AluOpType.add)
                                nc.scalar.activation(pS[:, 0:W], pS[:, 0:W],
                                                     AF.Exp)
                                p_tiles[(hh, it)] = pS
                        njc128 = 12 if ig == 0 else 16
                        for hh in range(2):
                            h = 2 * ppi + hh
                            avps = avp.tile([128, 512], f32, tag="av")
                            for jc in range(njc128):
                                pt = ptp.tile([128, 512], b16, tag="pT")
                                for itg in range(4):
                                    it = ig * 4 + itg
                                    d = pt[:, itg * 128:(itg + 1) * 128]
                                    if jc * 128 < _v(it) and not os.environ.get("NO_XBAR"):
                                        nc.sync.dma_start_transpose(
                                            d, p_tiles[(hh, it)][:, jc * 128: jc * 128 + 128])
                                    else:
                                        nc.gpsimd.memset(d, 0.0)
                                lhs = v_sb[:, jc * 272 + h * VPITCH:
                                           jc * 272 + h * VPITCH + 65]
                                nc.tensor.matmul(avps[0:65, :], lhs, pt[:],
                                                 start=(jc == 0),
                                                 stop=(jc == njc128 - 1))
                            rec_t = avsp.tile([128, 512], f32, tag="rec_t")
                            nc.vector.reciprocal(rec_t[64:65, :], avps[64:65, :])
                            rec0 = avsp.tile([1, 512], f32, tag="rec0")
                            nc.sync.dma_start(rec0[:], rec_t[64:65, :])
                            recb = avsp.tile([64, 512], f32, tag="recb")
                            if os.environ.get("NO_BCAST_AV"):
                                nc.gpsimd.memset(recb[:], 1.0)
                            else:
                                nc.gpsimd.partition_broadcast(recb[:], rec0[:])
                            dst_cols = slice(ppi * NTOK + ig * 512,
                                             ppi * NTOK + ig * 512 + 512)
                            if hh == 0:
                                nc.vector.tensor_mul(av_sb[0:64, dst_cols],
                                                     avps[0:64, :], recb[:])
                            else:
                                avt = avsp.tile([64, 512], b16, tag="avt")
                                nc.vector.tensor_mul(avt[:], avps[0:64, :],
                                                     recb[:])
                                nc.sync.dma_start(av_sb[64:128, dst_cols],
                                                  avt[:])

            # ---------------- phase C: o-proj + AllReduce + LN1 -------------
            with tc.tile_pool(name="wc", bufs=1) as wc, \
                 tc.tile_pool(name="pc", bufs=2, space="PSUM") as pc:
                oww = wc.tile([128, 2 * DM], b16, tag="oww")
                load_blocked(oww[:], owT[l], 2)
                obf = wc.tile([128, NBLK * NTOK], b16, tag="obf")
                for m in range(NBLK):
                    ps = pc.tile([128, 1024], f32, tag="pc")
                    for t2 in range(2):
                        for c in range(2):
                            nc.tensor.matmul(
                                ps[:, t2 * 512:(t2 + 1) * 512],
                                oww[:, c * DM + m * 128: c * DM + m * 128 + 128],
                                av_sb[:, c * NTOK + t2 * 512: c * NTOK + t2 * 512 + 512],
                                start=(c == 0), stop=(c == 1))
                    nc.scalar.activation(obf[:, m * NTOK:(m + 1) * NTOK], ps[:],
                                         AF.Identity, bias=bcol(l, 6 + m))
                cin = dram.tile([128, NBLK * NTOK], b16, tag="cin")
                cout = dram.tile([128, NBLK * NTOK], b16, tag="cout")
                nc.gpsimd.dma_start(cin[:], obf[:])
                nc.gpsimd.collective_compute(
                    "AllReduce", mybir.AluOpType.add,
                    replica_groups=GROUPS, ins=[cin.opt()], outs=[cout.opt()])
                ors = wc.tile([128, NBLK * NTOK], b16, tag="ors")
                nc.gpsimd.dma_start(ors[:], cout[:])
                nc.vector.tensor_add(hT[:], hT[:], ors[:])
                layer_norm(l, 22)

            # ---------------- phase D: FFN + AllReduce + LN2 ----------------
            with tc.tile_pool(name="wd", bufs=1) as wd, \
                 tc.tile_pool(name="pf", bufs=2, space="PSUM") as pf:
                f1w = wd.tile([128, NBLK * DIH], b16, tag="f1w")
                load_blocked(f1w[:], ff1T[l], NBLK)
                f2w = wd.tile([128, 8 * DM], b16, tag="f2w")
                load_blocked(f2w[:], ff2T[l], 8)
                ffa = wd.tile([128, 8 * NTOK], b16, tag="ffa")
                for m in range(8):
                    ps = pf.tile([128, 1024], f32, tag="pf")
                    for t2 in range(2):
                        for a in range(NBLK):
                            nc.tensor.matmul(
                                ps[:, t2 * 512:(t2 + 1) * 512],
                                f1w[:, a * DIH + m * 128: a * DIH + m * 128 + 128],
                                hTb[:, a * NTOK + t2 * 512: a * NTOK + t2 * 512 + 512],
                                start=(a == 0), stop=(a == NBLK - 1))
                    nc.scalar.activation(ffa[:, m * NTOK:(m + 1) * NTOK], ps[:],
                                         AF.Relu, bias=bcol(l, 10 + m))
                f2b = wd.tile([128, NBLK * NTOK], b16, tag="f2b")
                for m in range(NBLK):
                    ps = pf.tile([128, 1024], f32, tag="pf")
                    for t2 in range(2):
                        for c in range(8):
                            nc.tensor.matmul(
                                ps[:, t2 * 512:(t2 + 1) * 512],
                                f2w[:, c * DM + m * 128: c * DM + m * 128 + 128],
                                ffa[:, c * NTOK + t2 * 512: c * NTOK + t2 * 512 + 512],
                                start=(c == 0), stop=(c == 7))
                    nc.scalar.activation(f2b[:, m * NTOK:(m + 1) * NTOK], ps[:],
                                         AF.Identity, bias=bcol(l, 18 + m))
                cin = dram.tile([128, NBLK * NTOK], b16, tag="cin")
                cout = dram.tile([128, NBLK * NTOK], b16, tag="cout")
                nc.gpsimd.dma_start(cin[:], f2b[:])
                nc.gpsimd.collective_compute(
                    "AllReduce", mybir.AluOpType.add,
                    replica_groups=GROUPS, ins=[cin.opt()], outs=[cout.opt()])
                frs = wd.tile([128, NBLK * NTOK], b16, tag="frs")
                nc.gpsimd.dma_start(frs[:], cout[:])
                nc.vector.tensor_add(hT[:], hT[:], frs[:])
                layer_norm(l, 30)

        nc.sync.dma_start(yT.rearrange("(a p) t -> p a t", p=128),
                           hT[:].rearrange("p (a t) -> p a t", a=NBLK))

    return nc


# ======================= host side =======================

def _pack_core_inputs(inputs, core):
    b, g = core // 2, core % 2
    heads = list(range(HPC * g, HPC * g + HPC))
    f0 = HPC * g * DH
    fsl = slice(f0, f0 + 256)

    dec_inp = np.asarray(inputs["dec_inp"], np.float32)
    pos_emb = np.asarray(inputs["pos_emb"], np.float32)
    mems = np.asarray(inputs["mems"], np.float32)
    r_w_bias = np.asarray(inputs["r_w_bias"], np.float32)
    r_r_bias = np.asarray(inputs["r_r_bias"], np.float32)
    qkv_w = np.asarray(inputs["qkv_w"], np.float32)
    qkv_b = np.asarray(inputs["qkv_b"], np.float32)
    r_w = np.asarray(inputs["r_w"], np.float32)
    o_w = np.asarray(inputs["o_w"], np.float32)
    ff_w1 = np.asarray(inputs["ff_w1"], np.float32)
    ff_b1 = np.asarray(inputs["ff_b1"], np.float32)
    ff_w2 = np.asarray(inputs["ff_w2"], np.float32)
    ff_b2 = np.asarray(inputs["ff_b2"], np.float32)
    ln1_s = np.asarray(inputs["ln1_s"], np.float32)
    ln1_b = np.asarray(inputs["ln1_b"], np.float32)
    ln2_s = np.asarray(inputs["ln2_s"], np.float32)
    ln2_b = np.asarray(inputs["ln2_b"], np.float32)

    d = {}
    d["h0T"] = np.ascontiguousarray(dec_inp[:, b, :].T).astype(np.float32)
    d["memsT"] = np.ascontiguousarray(
        mems[:, :, b, :].transpose(0, 2, 1)).astype(bf16)
    d["posT"] = np.ascontiguousarray(pos_emb.T).astype(bf16)

    qkvwTa = np.empty((NL, DM, 768), np.float32)
    rwTa = np.empty((NL, DM, 256), np.float32)
    owTa = np.empty((NL, 256, DM), np.float32)
    f1a = np.empty((NL, DM, DIH), np.float32)
    f2a = np.empty((NL, DIH, DM), np.float32)
    biasA = np.zeros((128, 38 * NL), np.float32)
    for l in range(NL):
        qkvwTa[l, :, 0:256] = (qkv_w[l, fsl, :] * SC).T
        qkvwTa[l, :, 256:512] = qkv_w[l, DM + f0: DM + f0 + 256, :].T
        qkvwTa[l, :, 512:768] = qkv_w[l, 2 * DM + f0: 2 * DM + f0 + 256, :].T
        rwTa[l] = r_w[l, fsl, :].T
        owTa[l] = o_w[l][:, fsl].T
        f1a[l] = ff_w1[l, g * DIH:(g + 1) * DIH, :].T
        f2a[l] = ff_w2[l][:, g * DIH:(g + 1) * DIH].T

        base = 38 * l
        bq = qkv_b[l, fsl].reshape(HPC, DH)
        rwb = r_w_bias[heads, :]
        rrb = r_r_bias[heads, :]
        for ppi in range(PAIRS):
            qw = (bq[2 * ppi: 2 * ppi + 2] + rwb[2 * ppi: 2 * ppi + 2]) * SC
            qr = (bq[2 * ppi: 2 * ppi + 2] + rrb[2 * ppi: 2 * ppi + 2]) * SC
            biasA[:, base + 0 + ppi] = qw.reshape(128)
            biasA[:, base + 2 + ppi] = qr.reshape(128)
            biasA[:, base + 4 + ppi] = \
                qkv_b[l, DM + f0 + ppi * 128: DM + f0 + (ppi + 1) * 128]
        ob = o_w[l][:, fsl] @ qkv_b[l, 2 * DM + f0: 2 * DM + f0 + 256]
        fb1 = ff_b1[l, g * DIH:(g + 1) * DIH]
        for a in range(NBLK):
            biasA[:, base + 6 + a] = ob[a * 128:(a + 1) * 128]
            biasA[:, base + 18 + a] = ff_b2[l, a * 128:(a + 1) * 128] / 2.0
            biasA[:, base + 22 + a] = ln1_s[l, a * 128:(a + 1) * 128]
            biasA[:, base + 26 + a] = ln1_b[l, a * 128:(a + 1) * 128]
            biasA[:, base + 30 + a] = ln2_s[l, a * 128:(a + 1) * 128]
            biasA[:, base + 34 + a] = ln2_b[l, a * 128:(a + 1) * 128]
        for m in range(8):
            biasA[:, base + 10 + m] = fb1[m * 128:(m + 1) * 128]
    d["qkvwT"] = qkvwTa.astype(bf16)
    d["rwT"] = rwTa.astype(bf16)
    d["owT"] = owTa.astype(bf16)
    d["ff1T"] = f1a.astype(bf16)
    d["ff2T"] = f2a.astype(bf16)
    d["bias"] = biasA
    return d


def get_compiled():
    global _COMPILED
    if _COMPILED is None:
        nc = build_program()
        nc.finalize()
        _COMPILED = nc
    return _COMPILED


def run(inputs, trace=False, **kw):
    from concourse import bass_utils
    nc = get_compiled()
    in_maps = [_pack_core_inputs(inputs, c) for c in range(NCORES)]
    res = bass_utils.run_bass_kernel_spmd(
        nc, in_maps, core_ids=list(range(NCORES)), trace=trace, **kw)
    out = np.empty((QLEN, B, DM), np.float32)
    for b_ in range(B):
        out[:, b_, :] = res.results[2 * b_]["yT"].T
    return out, res


def kernel(**inputs):
    out, _ = run(inputs, trace=False)
    return out



# revision 7
# speedup vs baseline: 1.9931x; 1.9931x over previous
# Transformer-XL decoder (4 layers) on 8 trn2 NeuronCores.
# Sharding: core = (batch b, head-group g); b = core//2, g = core%2.
# Each core: 4 heads of attention for its batch + half of d_inner for the FFN.
# Pair cores (2b, 2b+1) AllReduce the o-projection and FF2 partial sums.
#
# Device layouts (per core):
#   activations feature-major, bf16 residual stream: hT [128, 4*1024]
#     (dm = a*128+p at col a*1024+t)
#   k/rk/q'/q'' per head-pair, rows hh*64+d (hh = head-in-pair, K=64 row-
#     packed matmuls with base partitions {0, 64})
#   v token-major: v_sb [128, 16*272]; window jw at col jw*272, head h at
#     +h*68 (64 cols + a ones col at +64 for the softmax denominator)
#   scores S[i-part, j-free]; BDpre[i-part, r-free], r = j - i + rlo offset.
#     Band width per q-tile it: W = 1152 + 128*it (both the valid j range
#     and the valid r range).  BD chunks evicted via ACT, AC via DVE; one
#     skewed SBUF->SBUF DMA with accum_op=add adds the rel-shifted BD onto
#     the evicted AC; 128 pad cols of -1e4 land on causally-masked j.
#   exp in place (ACT); ONE batched xbar dma_start_transpose per
#   (pair, ig, hh, it) with a 3D out AP writes all 128-col blocks of
#   P^T [j-part, i-free] interleaved into PT [128, njc*512]; AV with
#   lhsT=[v|1] (M=65, row 64 = denominator).

import os
import numpy as np
import ml_dtypes

NL, NH, DM, DH, DI = 4, 8, 512, 64, 2048
QLEN, MLEN, B = 1024, 1024, 4
KLEN = QLEN + MLEN
NCORES = 8
HPC = NH // 2        # heads per core
PAIRS = HPC // 2     # head pairs per core
DIH = DI // 2        # d_inner slice per core
SC = 1.0 / np.sqrt(DH)
NEG = -10000.0
NBLK = DM // 128
NTOK = QLEN
VPITCH = 68
TWBD = 2432          # bd tile width: max band 2048 + 128 pad, padded

bf16 = ml_dtypes.bfloat16

_COMPILED = None


def _w(it):
    return 1152 + 128 * it      # valid j width == valid r band width


def build_program(n_layers=NL):
    import concourse.bass as bass
    from concourse import bacc
    import concourse.tile as tile
    import concourse.mybir as mybir
    from concourse.ap import AP
    from concourse import library_config

    f32 = mybir.dt.float32
    b16 = mybir.dt.bfloat16
    AF = mybir.ActivationFunctionType
    GROUPS = [[0, 1], [2, 3], [4, 5], [6, 7]]

    nc = bacc.Bacc("TRN2", num_devices=NCORES, debug=False)

    h0T = nc.dram_tensor("h0T", [DM, NTOK], b16, kind="ExternalInput").ap()
    memsT = nc.dram_tensor("memsT", [NL, DM, MLEN], b16, kind="ExternalInput").ap()
    posT = nc.dram_tensor("posT", [DM, KLEN], b16, kind="ExternalInput").ap()
    qkvwT = nc.dram_tensor("qkvwT", [NL, DM, 768], b16, kind="ExternalInput").ap()
    rwT = nc.dram_tensor("rwT", [NL, DM, 256], b16, kind="ExternalInput").ap()
    owT = nc.dram_tensor("owT", [NL, 256, DM], b16, kind="ExternalInput").ap()
    ff1T = nc.dram_tensor("ff1T", [NL, DM, DIH], b16, kind="ExternalInput").ap()
    ff2T = nc.dram_tensor("ff2T", [NL, DIH, DM], b16, kind="ExternalInput").ap()
    bias = nc.dram_tensor("bias", [128, 38 * NL], f32, kind="ExternalInput").ap()
    yT = nc.dram_tensor("yT", [DM, NTOK], b16, kind="ExternalOutput").ap()

    # bias column map (keep in sync with _pack_core_inputs):
    # base = 38*l: +0,1 qb_w(pair) | +2,3 qb_r | +4,5 kb | +6..9 ob |
    # +10..17 ff1b | +18..21 ff2b/2 | +22..25 ln1_s | +26..29 ln1_b |
    # +30..33 ln2_s | +34..37 ln2_b

    with tile.TileContext(nc) as tc:
      with tc.tile_pool(name="persist", bufs=1) as pp_, \
           tc.tile_pool(name="dramp", bufs=2, space="DRAM") as dram:
        hT = pp_.tile([128, NBLK * NTOK], b16)
        posTs = pp_.tile([128, NBLK * KLEN], b16)
        bias_sb = pp_.tile([128, 38 * NL], f32)
        ones_c = pp_.tile([128, 1], b16)
        eps_c = pp_.tile([128, 1], f32)

        def load_blocked(dst_ap, src2d, nb):
            nc.sync.dma_start(dst_ap.rearrange("p (a t) -> p a t", a=nb),
                              src2d.rearrange("(a p) t -> p a t", p=128))

        def bcol(l, off):
            return bias_sb[:, 38 * l + off: 38 * l + off + 1]

        nc.gpsimd.load_library(library_config.mlp)
        nc.gpsimd.memset(ones_c[:], 1.0)
        nc.gpsimd.memset(eps_c[:], 1e-5)
        nc.sync.dma_start(bias_sb[:], bias)
        load_blocked(hT[:], h0T, NBLK)
        load_blocked(posTs[:], posT, NBLK)

        def layer_norm(l, boff):
            """LN over dm (partition blocks) of hT, in place (bf16)."""
            with tc.tile_pool(name="lns", bufs=1) as lp, \
                 tc.tile_pool(name="pl", bufs=2, space="PSUM") as pl:
                hsq = lp.tile([128, NBLK * 512], b16, tag="hsq")
                mu = lp.tile([1, NTOK], f32, tag="mu")
                ex2 = lp.tile([1, NTOK], f32, tag="ex2")
                tmp = lp.tile([1, NTOK], f32, tag="tmp")
                invc = lp.tile([1, 2 * NTOK], f32, tag="invc")
                invb = lp.tile([128, 2 * NTOK], f32, tag="invb")
                for t2 in range(2):
                    sx = pl.tile([128, 512], f32, tag="sx")
                    sq = pl.tile([128, 512], f32, tag="sq")
                    for a in range(NBLK):
                        seg = hT[:, a * NTOK + t2 * 512: a * NTOK + t2 * 512 + 512]
                        nc.tensor.matmul(sx[0:1, :], ones_c[:], seg,
                                         start=(a == 0), stop=(a == NBLK - 1))
                        hs = hsq[:, a * 512:(a + 1) * 512]
                        nc.scalar.activation(hs, seg, AF.Square)
                        nc.tensor.matmul(sq[0:1, :], ones_c[:], hs,
                                         start=(a == 0), stop=(a == NBLK - 1))
                    ts = slice(t2 * 512, t2 * 512 + 512)
                    nc.scalar.mul(mu[:, ts], sx[0:1, :], 1.0 / DM)
                    nc.scalar.mul(ex2[:, ts], sq[0:1, :], 1.0 / DM)
                nc.vector.tensor_mul(tmp[:], mu[:], mu[:])      # mu^2
                nc.vector.tensor_sub(ex2[:], ex2[:], tmp[:])    # var
                nc.scalar.activation(tmp[:], ex2[:], AF.Sqrt, bias=eps_c[0:1, :])
                inv = invc[:, 0:NTOK]
                cneg = invc[:, NTOK:2 * NTOK]
                if os.environ.get("V_EXACT_RECIP"):
                    nc.vector.reciprocal(inv, tmp[:])
                else:
                    nc.vector.reciprocal_approx_fast(inv, tmp[:])
                nc.vector.tensor_mul(cneg, mu[:], inv)
                nc.scalar.mul(cneg, cneg, -1.0)
                nc.gpsimd.partition_broadcast(invb[:], invc[:])
                for a in range(NBLK):
                    asl = slice(a * NTOK, (a + 1) * NTOK)
                    nc.vector.tensor_mul(hT[:, asl], hT[:, asl],
                                         invb[:, 0:NTOK])
                    nc.vector.tensor_add(hT[:, asl], hT[:, asl],
                                         invb[:, NTOK:2 * NTOK])
                    nc.scalar.activation(hT[:, asl], hT[:, asl], AF.Identity,
                                         bias=bcol(l, boff + 4 + a),
                                         scale=bcol(l, boff + a))

        for l in range(n_layers):
          with tc.tile_pool(name="wa", bufs=1) as wa, \
               tc.tile_pool(name="attn_in", bufs=1) as ai:
            memsL = wa.tile([128, NBLK * MLEN], b16)
            load_blocked(memsL[:], memsT[l], NBLK)
            qkvw = wa.tile([128, NBLK * 768], b16)
            load_blocked(qkvw[:], qkvwT[l], NBLK)
            rww = wa.tile([128, NBLK * 256], b16)
            load_blocked(rww[:], rwT[l], NBLK)

            k_sb = [ai.tile([128, KLEN], b16, tag=f"k_sb{i_}", name=f"k_sb{i_}") for i_ in range(PAIRS)]
            rk_sb = [ai.tile([128, KLEN], b16, tag=f"rk_sb{i_}", name=f"rk_sb{i_}") for i_ in range(PAIRS)]
            qw_sb = [ai.tile([128, NTOK], b16, tag=f"qw_sb{i_}", name=f"qw_sb{i_}") for i_ in range(PAIRS)]
            qr_sb = [ai.tile([128, NTOK], b16, tag=f"qr_sb{i_}", name=f"qr_sb{i_}") for i_ in range(PAIRS)]
            v_sb = ai.tile([128, 16 * 272], b16, tag="v_sb")
            av_sb = ai.tile([128, 2 * NTOK], b16, tag="av_sb")

            def cat_rhs(a, j0, n):
                if j0 < MLEN:
                    return memsL[:, a * MLEN + j0: a * MLEN + j0 + n]
                t0 = j0 - MLEN
                return hT[:, a * NTOK + t0: a * NTOK + t0 + n]

            # ---------------- phase A: qkv + rk ----------------
            with tc.tile_pool(name="pa", bufs=4, space="PSUM") as pa, \
                 tc.tile_pool(name="pv", bufs=2, space="PSUM") as pv:
                for ppi in range(PAIRS):
                    for jc in range(4):
                        ps = pa.tile([128, 512], f32, tag="pa")
                        for a in range(NBLK):
                            lhs = qkvw[:, a * 768 + 256 + ppi * 128:
                                       a * 768 + 256 + ppi * 128 + 128]
                            nc.tensor.matmul(ps[:], lhs,
                                             cat_rhs(a, jc * 512, 512),
                                             start=(a == 0), stop=(a == NBLK - 1))
                        nc.scalar.activation(
                            k_sb[ppi][:, jc * 512:(jc + 1) * 512], ps[:],
                            AF.Identity, bias=bcol(l, 4 + ppi))
                for ppi in range(PAIRS):
                    for jc in range(2):
                        ps = pa.tile([128, 512], f32, tag="pa")
                        for a in range(NBLK):
                            lhs = qkvw[:, a * 768 + ppi * 128:
                                       a * 768 + ppi * 128 + 128]
                            nc.tensor.matmul(
                                ps[:], lhs,
                                hT[:, a * NTOK + jc * 512: a * NTOK + jc * 512 + 512],
                                start=(a == 0), stop=(a == NBLK - 1))
                        nc.scalar.activation(
                            qw_sb[ppi][:, jc * 512:(jc + 1) * 512], ps[:],
                            AF.Identity, bias=bcol(l, 0 + ppi))
                        nc.scalar.activation(
                            qr_sb[ppi][:, jc * 512:(jc + 1) * 512], ps[:],
                            AF.Identity, bias=bcol(l, 2 + ppi))
                for ppi in range(PAIRS):
                    for jc in range(4):
                        ps = pa.tile([128, 512], f32, tag="pa")
                        for a in range(NBLK):
                            lhs = rww[:, a * 256 + ppi * 128:
                                      a * 256 + ppi * 128 + 128]
                            nc.tensor.matmul(
                                ps[:], lhs,
                                posTs[:, a * KLEN + jc * 512: a * KLEN + jc * 512 + 512],
                                start=(a == 0), stop=(a == NBLK - 1))
                        nc.vector.tensor_copy(
                            rk_sb[ppi][:, jc * 512:(jc + 1) * 512], ps[:])
                nc.gpsimd.memset(v_sb[:], 1.0)
                for jw in range(16):
                    ps = pv.tile([128, 256], f32, tag="pv")
                    for a in range(NBLK):
                        lhs = cat_rhs(a, jw * 128, 128)
                        nc.tensor.matmul(ps[:], lhs,
                                         qkvw[:, a * 768 + 512: a * 768 + 768],
                                         start=(a == 0), stop=(a == NBLK - 1))
                    dst = AP(v_sb[:].tensor, v_sb[:].offset + jw * 272,
                             [[16 * 272, 128], [VPITCH, HPC], [1, 64]])
                    nc.vector.tensor_copy(
                        dst, ps[:].rearrange("p (h d) -> p h d", d=64))

            # ---------------- phase B: attention ----------------
            with tc.tile_pool(name="bd", bufs=3) as bdp, \
                 tc.tile_pool(name="p_sb", bufs=9) as psb, \
                 tc.tile_pool(name="pT", bufs=2) as ptp, \
                 tc.tile_pool(name="avs", bufs=2) as avsp, \
                 tc.tile_pool(name="sc", bufs=5, space="PSUM") as scp, \
                 tc.tile_pool(name="av", bufs=2, space="PSUM") as avp:
                for ppi in range(PAIRS):
                  for ig in range(2):
                    NJCg = 12 if ig == 0 else 16
                    p_tiles = {}
                    for itg in range(4):
                        it = ig * 4 + itg
                        W = _w(it)            # valid j width == r band width
                        rlo = 896 - 128 * it
                        i0 = it * 128
                        njc = (W + 511) // 512
                        for hh in range(2):
                            hs = slice(hh * 64, hh * 64 + 64)
                            # BDpre [i, r], r in [rlo, 2048), evict via ACT
                            bd = bdp.tile([128, TWBD], b16, tag="bd")
                            for rc in range(njc):
                                n = min(512, W - rc * 512)
                                ps = scp.tile([128, 512], f32, tag="sc")
                                nc.tensor.matmul(
                                    ps[:, 0:n],
                                    qr_sb[ppi][hs, i0:i0 + 128],
                                    rk_sb[ppi][hs, rlo + rc * 512: rlo + rc * 512 + n],
                                    start=True, stop=True)
                                if os.environ.get("V_DVE_BD"):
                                    nc.vector.tensor_copy(
                                        bd[:, rc * 512: rc * 512 + n],
                                        ps[:, 0:n])
                                else:
                                    nc.scalar.activation(
                                        bd[:, rc * 512: rc * 512 + n],
                                        ps[:, 0:n], AF.Identity)
                            nc.gpsimd.memset(bd[:, W:W + 128], NEG)
                            # AC [i, j], j in [0, W), evict via DVE
                            pS = psb.tile([128, 2048], b16, tag="p_sb")
                            for jc in range(njc):
                                n = min(512, W - jc * 512)
                                ps = scp.tile([128, 512], f32, tag="sc")
                                nc.tensor.matmul(
                                    ps[:, 0:n],
                                    qw_sb[ppi][hs, i0:i0 + 128],
                                    k_sb[ppi][hs, jc * 512: jc * 512 + n],
                                    start=True, stop=True)
                                nc.vector.tensor_copy(
                                    pS[:, jc * 512: jc * 512 + n], ps[:, 0:n])
                            # skewed rel-shift add of BD onto AC
                            src = AP(bd[:].tensor, bd[:].offset + 127,
                                     [[TWBD - 1, 128], [1, W]])
                            nc.gpsimd.dma_start(pS[:, 0:W], src,
                                                accum_op=mybir.AluOpType.add)
                            nc.scalar.activation(pS[:, 0:W], pS[:, 0:W],
                                                 AF.Exp)
                            p_tiles[(hh, itg)] = pS
                    for hh in range(2):
                        h = 2 * ppi + hh
                        # batched xbar transposes: P^T blocks interleaved
                        # into PT [128 j, jc-major x (4 it x 128 i)]
                        PT = ptp.tile([128, NJCg * 512], b16, tag=f"PT{ig}")
                        teng = nc.sync if hh == 0 else nc.scalar
                        for itg in range(4):
                            it = ig * 4 + itg
                            W = _w(it)
                            nb = W // 128
                            if os.environ.get("V_SMALL_T"):
                                for jcb in range(nb):
                                    teng.dma_start_transpose(
                                        PT[:, jcb * 512 + itg * 128:
                                           jcb * 512 + itg * 128 + 128],
                                        p_tiles[(hh, itg)][:, jcb * 128:
                                                           jcb * 128 + 128])
                            else:
                                dst = AP(PT[:].tensor,
                                         PT[:].offset + itg * 128,
                                         [[NJCg * 512, 128], [512, nb],
                                          [1, 128]])
                                teng.dma_start_transpose(
                                    dst, p_tiles[(hh, itg)][:, 0:W])
                        for itg in range(4):
                            it = ig * 4 + itg
                            for jc in range(_w(it) // 128, NJCg):
                                nc.gpsimd.memset(
                                    PT[:, jc * 512 + itg * 128:
                                       jc * 512 + itg * 128 + 128], 0.0)
                        avps = avp.tile([128, 512], f32, tag="av")
                        for jc in range(NJCg):
                            lhs = v_sb[:, jc * 272 + h * VPITCH:
                                       jc * 272 + h * VPITCH + 65]
                            nc.tensor.matmul(avps[0:65, :], lhs,
                                             PT[:, jc * 512:(jc + 1) * 512],
                                             start=(jc == 0),
                                             stop=(jc == NJCg - 1))
                        rec_t = avsp.tile([128, 512], f32, tag="rec_t")
                        if os.environ.get("V_EXACT_RECIP"):
                            nc.vector.reciprocal(rec_t[64:65, :],
                                                 avps[64:65, :])
                        else:
                            nc.vector.tensor_copy(rec_t[64:65, :],
                                                  avps[64:65, :])
                            nc.vector.reciprocal_approx_fast(rec_t[64:65, :],
                                                             rec_t[64:65, :])
                        rec0 = avsp.tile([1, 512], f32, tag="rec0")
                        nc.sync.dma_start(rec0[:], rec_t[64:65, :])
                        recb = avsp.tile([64, 512], f32, tag="recb")
                        nc.gpsimd.partition_broadcast(recb[:], rec0[:])
                        dst_cols = slice(ppi * NTOK + ig * 512,
                                         ppi * NTOK + ig * 512 + 512)
                        if hh == 0:
                            nc.vector.tensor_mul(av_sb[0:64, dst_cols],
                                                 avps[0:64, :], recb[:])
                        else:
                            avt = avsp.tile([64, 512], b16, tag="avt")
                            nc.vector.tensor_mul(avt[:], avps[0:64, :],
                                                 recb[:])
                            nc.sync.dma_start(av_sb[64:128, dst_cols],
                                              avt[:])

            # ---------------- phase C: o-proj + AllReduce + LN1 -------------
            with tc.tile_pool(name="wc", bufs=1) as wc, \
                 tc.tile_pool(name="pc", bufs=2, space="PSUM") as pc:
                oww = wc.tile([128, 2 * DM], b16, tag="oww")
                load_blocked(oww[:], owT[l], 2)
                obf = wc.tile([128, NBLK * NTOK], b16, tag="obf")
                for m in range(NBLK):
                    ps = pc.tile([128, 1024], f32, tag="pc")
                    for t2 in range(2):
                        for c in range(2):
                            nc.tensor.matmul(
                                ps[:, t2 * 512:(t2 + 1) * 512],
                                oww[:, c * DM + m * 128: c * DM + m * 128 + 128],
                                av_sb[:, c * NTOK + t2 * 512: c * NTOK + t2 * 512 + 512],
                                start=(c == 0), stop=(c == 1))
                    nc.scalar.activation(obf[:, m * NTOK:(m + 1) * NTOK], ps[:],
                                         AF.Identity, bias=bcol(l, 6 + m))
                cin = dram.tile([128, NBLK * NTOK], b16, tag="cin")
                cout = dram.tile([128, NBLK * NTOK], b16, tag="cout")
                nc.gpsimd.dma_start(cin[:], obf[:])
                nc.gpsimd.collective_compute(
                    "AllReduce", mybir.AluOpType.add,
                    replica_groups=GROUPS, ins=[cin.opt()], outs=[cout.opt()])
                ors = wc.tile([128, NBLK * NTOK], b16, tag="ors")
                nc.gpsimd.dma_start(ors[:], cout[:])
                nc.vector.tensor_add(hT[:], hT[:], ors[:])
                layer_norm(l, 22)

            # ---------------- phase D: FFN + AllReduce + LN2 ----------------
            with tc.tile_pool(name="wd", bufs=1) as wd, \
                 tc.tile_pool(name="pf", bufs=2, space="PSUM") as pf:
                f1w = wd.tile([128, NBLK * DIH], b16, tag="f1w")
                load_blocked(f1w[:], ff1T[l], NBLK)
                f2w = wd.tile([128, 8 * DM], b16, tag="f2w")
                load_blocked(f2w[:], ff2T[l], 8)
                ffa = wd.tile([128, 8 * NTOK], b16, tag="ffa")
                for m in range(8):
                    ps = pf.tile([128, 1024], f32, tag="pf")
                    for t2 in range(2):
                        for a in range(NBLK):
                            nc.tensor.matmul(
                                ps[:, t2 * 512:(t2 + 1) * 512],
                                f1w[:, a * DIH + m * 128: a * DIH + m * 128 + 128],
                                hT[:, a * NTOK + t2 * 512: a * NTOK + t2 * 512 + 512],
                                start=(a == 0), stop=(a == NBLK - 1))
                    nc.scalar.activation(ffa[:, m * NTOK:(m + 1) * NTOK], ps[:],
                                         AF.Relu, bias=bcol(l, 10 + m))
                f2b = wd.tile([128, NBLK * NTOK], b16, tag="f2b")
                for m in range(NBLK):
                    ps = pf.tile([128, 1024], f32, tag="pf")
                    for t2 in range(2):
                        for c in range(8):
                            nc.tensor.matmul(
                                ps[:, t2 * 512:(t2 + 1) * 512],
                                f2w[:, c * DM + m * 128: c * DM + m * 128 + 128],
                                ffa[:, c * NTOK + t2 * 512: c * NTOK + t2 * 512 + 512],
                                start=(c == 0), stop=(c == 7))
                    nc.scalar.activation(f2b[:, m * NTOK:(m + 1) * NTOK], ps[:],
                                         AF.Identity, bias=bcol(l, 18 + m))
                cin = dram.tile([128, NBLK * NTOK], b16, tag="cin")
                cout = dram.tile([128, NBLK * NTOK], b16, tag="cout")
                nc.gpsimd.dma_start(cin[:], f2b[:])
                nc.gpsimd.collective_compute(
                    "AllReduce", mybir.AluOpType.add,
                    replica_groups=GROUPS, ins=[cin.opt()], outs=[cout.opt()])
                frs = wd.tile([128, NBLK * NTOK], b16, tag="frs")
                nc.gpsimd.dma_start(frs[:], cout[:])
                nc.vector.tensor_add(hT[:], hT[:], frs[:])
                layer_norm(l, 30)

        nc.sync.dma_start(yT.rearrange("(a p) t -> p a t", p=128),
                           hT[:].rearrange("p (a t) -> p a t", a=NBLK))

    return nc


# ======================= host side =======================

def _pack_core_inputs(inputs, core):
    b, g = core // 2, core % 2
    heads = list(range(HPC * g, HPC * g + HPC))
    f0 = HPC * g * DH
    fsl = slice(f0, f0 + 256)

    dec_inp = np.asarray(inputs["dec_inp"], np.float32)
    pos_emb = np.asarray(inputs["pos_emb"], np.float32)
    mems = np.asarray(inputs["mems"], np.float32)
    r_w_bias = np.asarray(inputs["r_w_bias"], np.float32)
    r_r_bias = np.asarray(inputs["r_r_bias"], np.float32)
    qkv_w = np.asarray(inputs["qkv_w"], np.float32)
    qkv_b = np.asarray(inputs["qkv_b"], np.float32)
    r_w = np.asarray(inputs["r_w"], np.float32)
    o_w = np.asarray(inputs["o_w"], np.float32)
    ff_w1 = np.asarray(inputs["ff_w1"], np.float32)
    ff_b1 = np.asarray(inputs["ff_b1"], np.float32)
    ff_w2 = np.asarray(inputs["ff_w2"], np.float32)
    ff_b2 = np.asarray(inputs["ff_b2"], np.float32)
    ln1_s = np.asarray(inputs["ln1_s"], np.float32)
    ln1_b = np.asarray(inputs["ln1_b"], np.float32)
    ln2_s = np.asarray(inputs["ln2_s"], np.float32)
    ln2_b = np.asarray(inputs["ln2_b"], np.float32)

    d = {}
    d["h0T"] = np.ascontiguousarray(dec_inp[:, b, :].T).astype(bf16)
    d["memsT"] = np.ascontiguousarray(
        mems[:, :, b, :].transpose(0, 2, 1)).astype(bf16)
    d["posT"] = np.ascontiguousarray(pos_emb.T).astype(bf16)

    qkvwTa = np.empty((NL, DM, 768), np.float32)
    rwTa = np.empty((NL, DM, 256), np.float32)
    owTa = np.empty((NL, 256, DM), np.float32)
    f1a = np.empty((NL, DM, DIH), np.float32)
    f2a = np.empty((NL, DIH, DM), np.float32)
    biasA = np.zeros((128, 38 * NL), np.float32)
    for l in range(NL):
        qkvwTa[l, :, 0:256] = (qkv_w[l, fsl, :] * SC).T
        qkvwTa[l, :, 256:512] = qkv_w[l, DM + f0: DM + f0 + 256, :].T
        qkvwTa[l, :, 512:768] = qkv_w[l, 2 * DM + f0: 2 * DM + f0 + 256, :].T
        rwTa[l] = r_w[l, fsl, :].T
        owTa[l] = o_w[l][:, fsl].T
        f1a[l] = ff_w1[l, g * DIH:(g + 1) * DIH, :].T
        f2a[l] = ff_w2[l][:, g * DIH:(g + 1) * DIH].T

        base = 38 * l
        bq = qkv_b[l, fsl].reshape(HPC, DH)
        rwb = r_w_bias[heads, :]
        rrb = r_r_bias[heads, :]
        for ppi in range(PAIRS):
            qw = (bq[2 * ppi: 2 * ppi + 2] + rwb[2 * ppi: 2 * ppi + 2]) * SC
            qr = (bq[2 * ppi: 2 * ppi + 2] + rrb[2 * ppi: 2 * ppi + 2]) * SC
            biasA[:, base + 0 + ppi] = qw.reshape(128)
            biasA[:, base + 2 + ppi] = qr.reshape(128)
            biasA[:, base + 4 + ppi] = \
                qkv_b[l, DM + f0 + ppi * 128: DM + f0 + (ppi + 1) * 128]
        ob = o_w[l][:, fsl] @ qkv_b[l, 2 * DM + f0: 2 * DM + f0 + 256]
        fb1 = ff_b1[l, g * DIH:(g + 1) * DIH]
        for a in range(NBLK):
            biasA[:, base + 6 + a] = ob[a * 128:(a + 1) * 128]
            biasA[:, base + 18 + a] = ff_b2[l, a * 128:(a + 1) * 128] / 2.0
            biasA[:, base + 22 + a] = ln1_s[l, a * 128:(a + 1) * 128]
            biasA[:, base + 26 + a] = ln1_b[l, a * 128:(a + 1) * 128]
            biasA[:, base + 30 + a] = ln2_s[l, a * 128:(a + 1) * 128]
            biasA[:, base + 34 + a] = ln2_b[l, a * 128:(a + 1) * 128]
        for m in range(8):
            biasA[:, base + 10 + m] = fb1[m * 128:(m + 1) * 128]
    d["qkvwT"] = qkvwTa.astype(bf16)
    d["rwT"] = rwTa.astype(bf16)
    d["owT"] = owTa.astype(bf16)
    d["ff1T"] = f1a.astype(bf16)
    d["ff2T"] = f2a.astype(bf16)
    d["bias"] = biasA
    return d


def get_compiled():
    global _COMPILED
    if _COMPILED is None:
        nc = build_program()
        nc.finalize()
        _COMPILED = nc
    return _COMPILED


def run(inputs, trace=False, **kw):
    from concourse import bass_utils
    nc = get_compiled()
    in_maps = [_pack_core_inputs(inputs, c) for c in range(NCORES)]
    res = bass_utils.run_bass_kernel_spmd(
        nc, in_maps, core_ids=list(range(NCORES)), trace=trace, **kw)
    out = np.empty((QLEN, B, DM), np.float32)
    for b_ in range(B):
        out[:, b_, :] = res.results[2 * b_]["yT"].astype(np.float32).T
    return out, res


def kernel(**inputs):
    out, _ = run(inputs, trace=False)
    return out


# revision 20
# speedup vs baseline: 2.3505x; 1.1794x over previous
# Transformer-XL decoder (4 layers) on 8 trn2 NeuronCores.
# Sharding: core = (batch b, head-group g); b = core//2, g = core%2.
# Attention: 4 heads per core over all 1024 query tokens.
# o-proj partials + 0.5*h are ReduceScattered over the pair -> each core
# owns a 512-token half (rank g) for LN1/FFN(full d_inner)/LN2, then an
# AllGather rebuilds the full residual stream for the next layer's QKV.
# The last layer skips the AllGather: each core stores only its own half
# and the host reassembles.  Next-layer weight loads + rk + K/V(mems)
# compute are emitted between the collective trigger and its consumer so
# they overlap the RS/AG latency.
#
# Device layouts (per core):
#   residual stream bf16, feature-major: hT [128, 4*1024]
#   scores S[i-part, j-free]; BDpre[i-part, r-free]; band width per q-tile
#   it: W = 1152 + 128*it.  BD/AC chunk matmuls interleaved across the two
#   heads of a pair (base partitions {0,64} -> PE row-packing); evictions
#   split ACT/DVE by head.  Skewed SBUF->SBUF DMA (accum add) applies the
#   rel-shift; 128 pad cols of -1e4 land on causally-masked j.
#   One batched xbar dma_start_transpose per (pair, ig, hh, it) with a 3D
#   out AP writes P^T blocks interleaved into PT [128, njc*512]; AV with
#   lhsT=[v|1] (M=65, row 64 = denominator).

import os
import numpy as np
import ml_dtypes

NL, NH, DM, DH, DI = 4, 8, 512, 64, 2048
QLEN, MLEN, B = 1024, 1024, 4
KLEN = QLEN + MLEN
NCORES = 8
HPC = NH // 2        # heads per core
PAIRS = HPC // 2     # head pairs per core
SC = 1.0 / np.sqrt(DH)
NEG = -10000.0
NBLK = DM // 128
NTOK = QLEN
HTOK = NTOK // 2     # own token half
VPITCH = 68
TWBD = 2432
NBC = 46             # bias cols per layer

bf16 = ml_dtypes.bfloat16

_COMPILED = None


def _w(it):
    return 1152 + 128 * it      # valid j width == valid r band width


def build_program(n_layers=NL):
    import concourse.bass as bass
    from concourse import bacc
    import concourse.tile as tile
    import concourse.mybir as mybir
    from concourse.ap import AP
    from concourse import library_config

    f32 = mybir.dt.float32
    b16 = mybir.dt.bfloat16
    AF = mybir.ActivationFunctionType
    GROUPS = [[0, 1], [2, 3], [4, 5], [6, 7]]

    nc = bacc.Bacc("TRN2", num_devices=NCORES, debug=False)

    h0T = nc.dram_tensor("h0T", [DM, NTOK], b16, kind="ExternalInput").ap()
    memsT = nc.dram_tensor("memsT", [NL, DM, MLEN], b16, kind="ExternalInput").ap()
    posT = nc.dram_tensor("posT", [DM, KLEN], b16, kind="ExternalInput").ap()
    qkvwT = nc.dram_tensor("qkvwT", [NL, DM, 768], b16, kind="ExternalInput").ap()
    rwT = nc.dram_tensor("rwT", [NL, DM, 256], b16, kind="ExternalInput").ap()
    owT = nc.dram_tensor("owT", [NL, 256, DM], b16, kind="ExternalInput").ap()
    ff1T = nc.dram_tensor("ff1T", [NL, DM, DI], b16, kind="ExternalInput").ap()
    ff2T = nc.dram_tensor("ff2T", [NL, DI, DM], b16, kind="ExternalInput").ap()
    bias = nc.dram_tensor("bias", [128, NBC * NL], f32, kind="ExternalInput").ap()
    yT = nc.dram_tensor("yT", [DM, HTOK], b16, kind="ExternalOutput").ap()
    dbgAV = nc.dram_tensor("dbgAV", [128, 2 * NTOK], b16, kind="ExternalOutput").ap()
    dbgHO = nc.dram_tensor("dbgHO", [128, NBLK * 512], b16, kind="ExternalOutput").ap()

    # bias column map (keep in sync with _pack_core_inputs):
    # base = NBC*l: +0,1 qb_w(pair) | +2,3 qb_r | +4,5 kb | +6..9 ob |
    # +10..25 ff1b | +26..29 ff2b | +30..33 ln1_s | +34..37 ln1_b |
    # +38..41 ln2_s | +42..45 ln2_b

    with tile.TileContext(nc) as tc:
      with tc.tile_pool(name="persist", bufs=1) as pp_, \
           tc.tile_pool(name="wa", bufs=1) as wa, \
           tc.tile_pool(name="attn_in", bufs=1) as ai, \
           tc.tile_pool(name="dramp", bufs=2, space="DRAM") as dram:
        hT = pp_.tile([128, NBLK * NTOK], b16)
        posTs = pp_.tile([128, NBLK * KLEN], b16)
        bias_sb = pp_.tile([128, NBC * NL], f32)
        ones_c = pp_.tile([128, 1], b16)
        eps_c = pp_.tile([128, 1], f32)

        def load_blocked(dst_ap, src2d, nb):
            nc.sync.dma_start(dst_ap.rearrange("p (a t) -> p a t", a=nb),
                              src2d.rearrange("(a p) t -> p a t", p=128))

        def bcol(l, off):
            return bias_sb[:, NBC * l + off: NBC * l + off + 1]

        nc.gpsimd.load_library(library_config.mlp)
        nc.gpsimd.memset(ones_c[:], 1.0)
        nc.gpsimd.memset(eps_c[:], 1e-5)
        nc.sync.dma_start(bias_sb[:], bias)
        load_blocked(hT[:], h0T, NBLK)
        load_blocked(posTs[:], posT, NBLK)

        # per-layer tiles allocated from hoisted pools (bufs=1: WAR deps
        # serialize only against the previous layer's finished readers)
        def layer_tiles(l):
            t = {}
            t["memsL"] = wa.tile([128, NBLK * MLEN], b16, tag="memsL", name="memsL")
            t["qkvw"] = wa.tile([128, NBLK * 768], b16, tag="qkvw", name="qkvw")
            t["rww"] = wa.tile([128, NBLK * 256], b16, tag="rww", name="rww")
            t["k"] = [ai.tile([128, KLEN], b16, tag=f"k_sb{i}", name=f"k_sb{i}") for i in range(PAIRS)]
            t["rk"] = [ai.tile([128, KLEN], b16, tag=f"rk_sb{i}", name=f"rk_sb{i}") for i in range(PAIRS)]
            t["qw"] = [ai.tile([128, NTOK], b16, tag=f"qw_sb{i}", name=f"qw_sb{i}") for i in range(PAIRS)]
            t["qr"] = [ai.tile([128, NTOK], b16, tag=f"qr_sb{i}", name=f"qr_sb{i}") for i in range(PAIRS)]
            t["v"] = ai.tile([128, 16 * 272], b16, tag="v_sb", name="v_sb")
            t["av"] = ai.tile([128, 2 * NTOK], b16, tag="av_sb", name="av_sb")
            return t

        def cat_rhs(t, a, j0, n):
            if j0 < MLEN:
                return t["memsL"][:, a * MLEN + j0: a * MLEN + j0 + n]
            tt = j0 - MLEN
            return hT[:, a * NTOK + tt: a * NTOK + tt + n]

        def prefetch_1(l, t, pool):
            """weight loads + rk for layer l (no dependence on h)."""
            load_blocked(t["memsL"][:], memsT[l], NBLK)
            load_blocked(t["qkvw"][:], qkvwT[l], NBLK)
            load_blocked(t["rww"][:], rwT[l], NBLK)
            for ppi in range(PAIRS):
                for jc in range(4):
                    ps = pool.tile([128, 512], f32, tag="pa")
                    for a in range(NBLK):
                        lhs = t["rww"][:, a * 256 + ppi * 128:
                                       a * 256 + ppi * 128 + 128]
                        nc.tensor.matmul(
                            ps[:], lhs,
                            posTs[:, a * KLEN + jc * 512: a * KLEN + jc * 512 + 512],
                            start=(a == 0), stop=(a == NBLK - 1))
                    nc.vector.tensor_copy(
                        t["rk"][ppi][:, jc * 512:(jc + 1) * 512], ps[:])

        def prefetch_2(l, t, pool):
            """K/V over the mems half (j < 1024)."""
            for ppi in range(PAIRS):
                for jc in range(2):
                    ps = pool.tile([128, 512], f32, tag="pa")
                    for a in range(NBLK):
                        lhs = t["qkvw"][:, a * 768 + 256 + ppi * 128:
                                        a * 768 + 256 + ppi * 128 + 128]
                        nc.tensor.matmul(ps[:], lhs,
                                         cat_rhs(t, a, jc * 512, 512),
                                         start=(a == 0), stop=(a == NBLK - 1))
                    nc.scalar.activation(
                        t["k"][ppi][:, jc * 512:(jc + 1) * 512], ps[:],
                        AF.Identity, bias=bcol(l, 4 + ppi))
            nc.gpsimd.memset(t["v"][:], 1.0)
            for jw in range(8):
                ps = pool.tile([128, 512], f32, tag="pa")
                for a in range(NBLK):
                    lhs = cat_rhs(t, a, jw * 128, 128)
                    nc.tensor.matmul(ps[:, 0:256], lhs,
                                     t["qkvw"][:, a * 768 + 512: a * 768 + 768],
                                     start=(a == 0), stop=(a == NBLK - 1))
                dst = AP(t["v"][:].tensor, t["v"][:].offset + jw * 272,
                         [[16 * 272, 128], [VPITCH, HPC], [1, 64]])
                nc.vector.tensor_copy(
                    dst, ps[:, 0:256].rearrange("p (h d) -> p h d", d=64))

        def phase_a_h(l, t, pool):
            """Q + K/V over the h half (needs rebuilt hT)."""
            for ppi in range(PAIRS):
                for jc in range(2):
                    ps = pool.tile([128, 512], f32, tag="pa")
                    for a in range(NBLK):
                        lhs = t["qkvw"][:, a * 768 + ppi * 128:
                                        a * 768 + ppi * 128 + 128]
                        nc.tensor.matmul(
                            ps[:], lhs,
                            hT[:, a * NTOK + jc * 512: a * NTOK + jc * 512 + 512],
                            start=(a == 0), stop=(a == NBLK - 1))
                    nc.scalar.activation(
                        t["qw"][ppi][:, jc * 512:(jc + 1) * 512], ps[:],
                        AF.Identity, bias=bcol(l, 0 + ppi))
                    nc.scalar.activation(
                        t["qr"][ppi][:, jc * 512:(jc + 1) * 512], ps[:],
                        AF.Identity, bias=bcol(l, 2 + ppi))
                for jc in range(2, 4):
                    ps = pool.tile([128, 512], f32, tag="pa")
                    for a in range(NBLK):
                        lhs = t["qkvw"][:, a * 768 + 256 + ppi * 128:
                                        a * 768 + 256 + ppi * 128 + 128]
                        nc.tensor.matmul(ps[:], lhs,
                                         cat_rhs(t, a, jc * 512, 512),
                                         start=(a == 0), stop=(a == NBLK - 1))
                    nc.scalar.activation(
                        t["k"][ppi][:, jc * 512:(jc + 1) * 512], ps[:],
                        AF.Identity, bias=bcol(l, 4 + ppi))
            for jw in range(8, 16):
                ps = pool.tile([128, 512], f32, tag="pa")
                for a in range(NBLK):
                    lhs = cat_rhs(t, a, jw * 128, 128)
                    nc.tensor.matmul(ps[:, 0:256], lhs,
                                     t["qkvw"][:, a * 768 + 512: a * 768 + 768],
                                     start=(a == 0), stop=(a == NBLK - 1))
                dst = AP(t["v"][:].tensor, t["v"][:].offset + jw * 272,
                         [[16 * 272, 128], [VPITCH, HPC], [1, 64]])
                nc.vector.tensor_copy(
                    dst, ps[:, 0:256].rearrange("p (h d) -> p h d", d=64))

        def layer_norm_own(l, hOwn, boff, lp, pl):
            """LN over dm of the own-token-half buffer hOwn, in place."""
            hsq = lp.tile([128, NBLK * 512], b16, tag="hsq")
            mu = lp.tile([1, HTOK], f32, tag="mu")
            ex2 = lp.tile([1, HTOK], f32, tag="ex2")
            tmp = lp.tile([1, HTOK], f32, tag="tmp")
            invc = lp.tile([1, 2 * HTOK], f32, tag="invc")
            invb = lp.tile([128, 2 * HTOK], f32, tag="invb")
            sx = pl.tile([128, 512], f32, tag="sx")
            sq = pl.tile([128, 512], f32, tag="sq")
            for a in range(NBLK):
                seg = hOwn[:, a * 512:(a + 1) * 512]
                nc.tensor.matmul(sx[0:1, :], ones_c[:], seg,
                                 start=(a == 0), stop=(a == NBLK - 1))
                hs = hsq[:, a * 512:(a + 1) * 512]
                nc.scalar.activation(hs, seg, AF.Square)
                nc.tensor.matmul(sq[0:1, :], ones_c[:], hs,
                                 start=(a == 0), stop=(a == NBLK - 1))
            nc.scalar.mul(mu[:], sx[0:1, :], 1.0 / DM)
            nc.scalar.mul(ex2[:], sq[0:1, :], 1.0 / DM)
            nc.vector.tensor_mul(tmp[:], mu[:], mu[:])
            nc.vector.tensor_sub(ex2[:], ex2[:], tmp[:])
            nc.scalar.activation(tmp[:], ex2[:], AF.Sqrt, bias=eps_c[0:1, :])
            inv = invc[:, 0:HTOK]
            cneg = invc[:, HTOK:2 * HTOK]
            if os.environ.get("V_EXACT_RECIP"):
                nc.vector.reciprocal(inv, tmp[:])
            else:
                nc.vector.reciprocal_approx_fast(inv, tmp[:])
            nc.vector.tensor_mul(cneg, mu[:], inv)
            nc.scalar.mul(cneg, cneg, -1.0)
            nc.gpsimd.partition_broadcast(invb[:], invc[:])
            for a in range(NBLK):
                asl = slice(a * 512, (a + 1) * 512)
                nc.vector.tensor_mul(hOwn[:, asl], hOwn[:, asl],
                                     invb[:, 0:HTOK])
                nc.vector.tensor_add(hOwn[:, asl], hOwn[:, asl],
                                     invb[:, HTOK:2 * HTOK])
                nc.scalar.activation(hOwn[:, asl], hOwn[:, asl], AF.Identity,
                                     bias=bcol(l, boff + 4 + a),
                                     scale=bcol(l, boff + a))

        # ---------------- layer 0 phase A ----------------
        tiles = layer_tiles(0)
        with tc.tile_pool(name="p0", bufs=3, space="PSUM") as p0:
            prefetch_1(0, tiles, p0)
            prefetch_2(0, tiles, p0)
            phase_a_h(0, tiles, p0)

        for l in range(n_layers):
          t = tiles

          # ---------------- phase B: attention ----------------
          with tc.tile_pool(name="bd", bufs=4) as bdp, \
               tc.tile_pool(name="p_sb", bufs=7) as psb, \
               tc.tile_pool(name="pT", bufs=2) as ptp, \
               tc.tile_pool(name="avs", bufs=2) as avsp, \
               tc.tile_pool(name="sc", bufs=6, space="PSUM") as scp, \
               tc.tile_pool(name="av", bufs=2, space="PSUM") as avp:
            for ppi in range(PAIRS):
              for ig in range(2):
                NJCg = 12 if ig == 0 else 16
                PTs = [ptp.tile([128, 16 * 512], b16, tag="PT",
                                 name=f"PT{ppi}_{ig}_{hh_}")
                       for hh_ in range(2)]
                for itg in range(4):
                    it = ig * 4 + itg
                    W = _w(it)
                    rlo = 896 - 128 * it
                    i0 = it * 128
                    njc = (W + 511) // 512
                    hsl = [slice(0, 64), slice(64, 128)]
                    # BD, heads interleaved for PE row packing
                    bd2 = [bdp.tile([128, TWBD], b16, tag="bd",
                                     name=f"bd{it}_{hh_}")
                           for hh_ in range(2)]
                    for rc in range(njc):
                        n = min(512, W - rc * 512)
                        pshh = []
                        for hh in range(2):
                            ps = scp.tile([128, 512], f32, tag="sc",
                                           name=f"sc{it}_{hh}")
                            nc.tensor.matmul(
                                ps[:, 0:n],
                                t["qr"][ppi][hsl[hh], i0:i0 + 128],
                                t["rk"][ppi][hsl[hh],
                                             rlo + rc * 512: rlo + rc * 512 + n],
                                start=True, stop=True)
                            pshh.append(ps)
                        nc.scalar.activation(
                            bd2[0][:, rc * 512: rc * 512 + n],
                            pshh[0][:, 0:n], AF.Identity)
                        nc.vector.tensor_copy(
                            bd2[1][:, rc * 512: rc * 512 + n],
                            pshh[1][:, 0:n])
                    for hh in range(2):
                        nc.gpsimd.memset(bd2[hh][:, W:W + 128], NEG)
                    # AC, heads interleaved
                    pS2 = [psb.tile([128, 2048], b16, tag="p_sb",
                                     name=f"pS{it}_{hh_}")
                           for hh_ in range(2)]
                    for jc in range(njc):
                        n = min(512, W - jc * 512)
                        pshh = []
                        for hh in range(2):
                            ps = scp.tile([128, 512], f32, tag="sc",
                                           name=f"sc{it}_{hh}")
                            nc.tensor.matmul(
                                ps[:, 0:n],
                                t["qw"][ppi][hsl[hh], i0:i0 + 128],
                                t["k"][ppi][hsl[hh], jc * 512: jc * 512 + n],
                                start=True, stop=True)
                            pshh.append(ps)
                        nc.vector.tensor_copy(
                            pS2[0][:, jc * 512: jc * 512 + n], pshh[0][:, 0:n])
                        nc.scalar.activation(
                            pS2[1][:, jc * 512: jc * 512 + n],
                            pshh[1][:, 0:n], AF.Identity)
                    for hh in range(2):
                        src = AP(bd2[hh][:].tensor, bd2[hh][:].offset + 127,
                                 [[TWBD - 1, 128], [1, W]])
                        nc.gpsimd.dma_start(pS2[hh][:, 0:W], src,
                                            accum_op=mybir.AluOpType.add)
                        nc.scalar.activation(pS2[hh][:, 0:W], pS2[hh][:, 0:W],
                                             AF.Exp)
                        # batched transpose of P^T blocks into PT
                        nb = W // 128
                        dst = AP(PTs[hh][:].tensor,
                                 PTs[hh][:].offset + itg * 128,
                                 [[16 * 512, 128], [512, nb], [1, 128]])
                        teng = nc.sync if hh == 0 else nc.scalar
                        teng.dma_start_transpose(dst, pS2[hh][:, 0:W])
                for hh in range(2):
                    h = 2 * ppi + hh
                    PT = PTs[hh]
                    for itg in range(4):
                        it = ig * 4 + itg
                        for jc in range(_w(it) // 128, NJCg):
                            nc.gpsimd.memset(
                                PT[:, jc * 512 + itg * 128:
                                   jc * 512 + itg * 128 + 128], 0.0)
                    avps = avp.tile([128, 512], f32, tag="av")
                    for jc in range(NJCg):
                        lhs = t["v"][:, jc * 272 + h * VPITCH:
                                     jc * 272 + h * VPITCH + 65]
                        nc.tensor.matmul(avps[0:65, :], lhs,
                                         PT[:, jc * 512:(jc + 1) * 512],
                                         start=(jc == 0),
                                         stop=(jc == NJCg - 1))
                    rec_t = avsp.tile([128, 1024], f32, tag="rec_t")
                    if os.environ.get("V_EXACT_RECIP"):
                        nc.vector.reciprocal(rec_t[64:65, 512:1024],
                                             avps[64:65, :])
                    else:
                        nc.vector.tensor_copy(rec_t[64:65, 0:512],
                                              avps[64:65, :])
                        nc.vector.reciprocal_approx_fast(
                            rec_t[64:65, 512:1024], rec_t[64:65, 0:512])
                    rec0 = avsp.tile([1, 512], f32, tag="rec0")
                    nc.sync.dma_start(rec0[:], rec_t[64:65, 512:1024])
                    recb = avsp.tile([64, 512], f32, tag="recb")
                    nc.gpsimd.partition_broadcast(recb[:], rec0[:])
                    dst_cols = slice(ppi * NTOK + ig * 512,
                                     ppi * NTOK + ig * 512 + 512)
                    if hh == 0:
                        nc.vector.tensor_mul(t["av"][0:64, dst_cols],
                                             avps[0:64, :], recb[:])
                    else:
                        avt = avsp.tile([64, 512], b16, tag="avt")
                        nc.vector.tensor_mul(avt[:], avps[0:64, :],
                                             recb[:])
                        nc.sync.dma_start(t["av"][64:128, dst_cols],
                                          avt[:])

          # ------- phase C: o-proj + ReduceScatter(0.5h + o) + LN1 -------
          # obf is packed th-major ([th0 all-dm | th1 all-dm]) so the RS
          # bounce is one 4KB-chunk contiguous DMA per (p, th).
          with tc.tile_pool(name="wc", bufs=1) as wc, \
               tc.tile_pool(name="lns", bufs=1) as lp:
            with tc.tile_pool(name="pc", bufs=2, space="PSUM") as pc:
                oww = wc.tile([128, 2 * DM], b16, tag="oww")
                load_blocked(oww[:], owT[l], 2)
                obf = wc.tile([128, NBLK * NTOK], b16, tag="obf")
                for m in range(NBLK):
                    for t2 in range(2):
                        ps = pc.tile([128, 512], f32, tag="pc")
                        for c in range(2):
                            nc.tensor.matmul(
                                ps[:],
                                oww[:, c * DM + m * 128: c * DM + m * 128 + 128],
                                t["av"][:, c * NTOK + t2 * 512:
                                        c * NTOK + t2 * 512 + 512],
                                start=(c == 0), stop=(c == 1))
                        # obf = (0.5*h + ob_bias) + o_partial
                        osl = slice(t2 * 2048 + m * 512,
                                    t2 * 2048 + m * 512 + 512)
                        hsl_ = slice(m * NTOK + t2 * 512,
                                     m * NTOK + t2 * 512 + 512)
                        if os.environ.get("V_NO_AFFINE"):
                            hh_t = wc.tile([128, 512], f32, tag="hhalf",
                                           name="hhalf")
                            nc.scalar.activation(obf[:, osl], ps[:],
                                                 AF.Identity,
                                                 bias=bcol(l, 6 + m))
                            nc.scalar.mul(hh_t[:], hT[:, hsl_], 0.5)
                            nc.vector.tensor_add(obf[:, osl], obf[:, osl],
                                                 hh_t[:])
                        else:
                            nc.vector.affine_then_add(
                                obf[:, osl], hT[:, hsl_],
                                ps[:], scale=0.5, bias=bcol(l, 6 + m))
                cinr = dram.tile([1, 2 * 128 * NBLK * 512], b16, tag="cinr")
                coutr = dram.tile([1, 128 * NBLK * 512], b16, tag="coutr")
                nc.sync.dma_start(
                    cinr[:].rearrange("o (th p x) -> o p th x", th=2, p=128),
                    obf[:].rearrange("p (th x) -> p th x", th=2))
                nc.gpsimd.collective_compute(
                    "ReduceScatter", mybir.AluOpType.add,
                    replica_groups=GROUPS, ins=[cinr.opt()],
                    outs=[coutr.opt()])
                # ---- prefetch for layer l+1 (independent of the RS) ----
                if l + 1 < n_layers:
                    tiles = layer_tiles(l + 1)
                    prefetch_1(l + 1, tiles, pc)
                if l == 0 and os.environ.get("V_DBG"):
                    nc.sync.dma_start(dbgAV, t["av"][:])
                hOwn = wc.tile([128, NBLK * 512], b16, tag="hOwn")
                nc.sync.dma_start(
                    hOwn[:], coutr[:].rearrange("o (p x) -> o p x", p=128))
                if l == 0 and os.environ.get("V_DBG"):
                    nc.sync.dma_start(dbgHO, hOwn[:])
                layer_norm_own(l, hOwn, 30, lp, pc)

            # ---------------- phase D: FFN + LN2 + AllGather -------------
            with tc.tile_pool(name="wd", bufs=1) as wd, \
                 tc.tile_pool(name="pf", bufs=2, space="PSUM") as pf:
                f1w = wd.tile([128, NBLK * DI], b16, tag="f1w")
                load_blocked(f1w[:], ff1T[l], NBLK)
                f2w = wd.tile([128, 16 * DM], b16, tag="f2w")
                load_blocked(f2w[:], ff2T[l], 16)
                ffa = wd.tile([128, 16 * 512], b16, tag="ffa")
                for m in range(16):
                    ps = pf.tile([128, 512], f32, tag="pf")
                    for a in range(NBLK):
                        nc.tensor.matmul(
                            ps[:],
                            f1w[:, a * DI + m * 128: a * DI + m * 128 + 128],
                            hOwn[:, a * 512:(a + 1) * 512],
                            start=(a == 0), stop=(a == NBLK - 1))
                    nc.scalar.activation(ffa[:, m * 512:(m + 1) * 512], ps[:],
                                         AF.Relu, bias=bcol(l, 10 + m))
                hOwn2 = wd.tile([128, NBLK * 512], b16, tag="hOwn2")
                for m in range(NBLK):
                    ps = pf.tile([128, 512], f32, tag="pf")
                    for c in range(16):
                        nc.tensor.matmul(
                            ps[:],
                            f2w[:, c * DM + m * 128: c * DM + m * 128 + 128],
                            ffa[:, c * 512:(c + 1) * 512],
                            start=(c == 0), stop=(c == 15))
                    # hOwn2 = (hOwn + ff2_bias) + ff2_partial
                    if os.environ.get("V_NO_AFFINE"):
                        nc.scalar.activation(
                            hOwn2[:, m * 512:(m + 1) * 512], ps[:],
                            AF.Identity, bias=bcol(l, 26 + m))
                        nc.vector.tensor_add(
                            hOwn2[:, m * 512:(m + 1) * 512],
                            hOwn2[:, m * 512:(m + 1) * 512],
                            hOwn[:, m * 512:(m + 1) * 512])
                    else:
                        nc.vector.affine_then_add(
                            hOwn2[:, m * 512:(m + 1) * 512],
                            hOwn[:, m * 512:(m + 1) * 512], ps[:],
                            scale=1.0, bias=bcol(l, 26 + m))
                layer_norm_own(l, hOwn2, 38, lp, pf)
                if l + 1 < n_layers:
                    # ---- prefetch K/V(mems) for l+1; its PE/DVE work
                    # overlaps the AllGather below ----
                    prefetch_2(l + 1, tiles, pf)
                    cing = dram.tile([1, 128 * NBLK * 512], b16, tag="cing")
                    coutg = dram.tile([1, 2 * 128 * NBLK * 512], b16,
                                      tag="coutg")
                    nc.sync.dma_start(
                        cing[:].rearrange("o (p x) -> o p x", p=128),
                        hOwn2[:])
                    nc.gpsimd.collective_compute(
                        "AllGather", mybir.AluOpType.bypass,
                        replica_groups=GROUPS,
                        ins=[cing.opt()], outs=[coutg.opt()])
                    # rebuild the full residual stream (3-dim DMA per half)
                    for th in range(2):
                        dstT = AP(hT[:].tensor, hT[:].offset + th * 512,
                                  [[NBLK * NTOK, 128], [NTOK, NBLK], [1, 512]])
                        srcT = AP(coutg[:].tensor,
                                  coutg[:].offset + th * 128 * NBLK * 512,
                                  [[NBLK * 512, 128], [512, NBLK], [1, 512]])
                        nc.sync.dma_start(dstT, srcT)
                    phase_a_h(l + 1, tiles, pf)
                else:
                    nc.sync.dma_start(
                        yT.rearrange("(a p) t -> p a t", p=128),
                        hOwn2[:].rearrange("p (a t) -> p a t", a=NBLK))

    return nc


# ======================= host side =======================

def _pack_core_inputs(inputs, core):
    b, g = core // 2, core % 2
    heads = list(range(HPC * g, HPC * g + HPC))
    f0 = HPC * g * DH
    fsl = slice(f0, f0 + 256)

    dec_inp = np.asarray(inputs["dec_inp"], np.float32)
    pos_emb = np.asarray(inputs["pos_emb"], np.float32)
    mems = np.asarray(inputs["mems"], np.float32)
    r_w_bias = np.asarray(inputs["r_w_bias"], np.float32)
    r_r_bias = np.asarray(inputs["r_r_bias"], np.float32)
    qkv_w = np.asarray(inputs["qkv_w"], np.float32)
    qkv_b = np.asarray(inputs["qkv_b"], np.float32)
    r_w = np.asarray(inputs["r_w"], np.float32)
    o_w = np.asarray(inputs["o_w"], np.float32)
    ff_w1 = np.asarray(inputs["ff_w1"], np.float32)
    ff_b1 = np.asarray(inputs["ff_b1"], np.float32)
    ff_w2 = np.asarray(inputs["ff_w2"], np.float32)
    ff_b2 = np.asarray(inputs["ff_b2"], np.float32)
    ln1_s = np.asarray(inputs["ln1_s"], np.float32)
    ln1_b = np.asarray(inputs["ln1_b"], np.float32)
    ln2_s = np.asarray(inputs["ln2_s"], np.float32)
    ln2_b = np.asarray(inputs["ln2_b"], np.float32)

    d = {}
    d["h0T"] = np.ascontiguousarray(dec_inp[:, b, :].T).astype(bf16)
    d["memsT"] = np.ascontiguousarray(
        mems[:, :, b, :].transpose(0, 2, 1)).astype(bf16)
    d["posT"] = np.ascontiguousarray(pos_emb.T).astype(bf16)

    qkvwTa = np.empty((NL, DM, 768), np.float32)
    rwTa = np.empty((NL, DM, 256), np.float32)
    owTa = np.empty((NL, 256, DM), np.float32)
    f1a = np.empty((NL, DM, DI), np.float32)
    f2a = np.empty((NL, DI, DM), np.float32)
    biasA = np.zeros((128, NBC * NL), np.float32)
    for l in range(NL):
        qkvwTa[l, :, 0:256] = (qkv_w[l, fsl, :] * SC).T
        qkvwTa[l, :, 256:512] = qkv_w[l, DM + f0: DM + f0 + 256, :].T
        qkvwTa[l, :, 512:768] = qkv_w[l, 2 * DM + f0: 2 * DM + f0 + 256, :].T
        rwTa[l] = r_w[l, fsl, :].T
        owTa[l] = o_w[l][:, fsl].T
        f1a[l] = ff_w1[l].T
        f2a[l] = ff_w2[l].T

        base = NBC * l
        bq = qkv_b[l, fsl].reshape(HPC, DH)
        rwb = r_w_bias[heads, :]
        rrb = r_r_bias[heads, :]
        for ppi in range(PAIRS):
            qw = (bq[2 * ppi: 2 * ppi + 2] + rwb[2 * ppi: 2 * ppi + 2]) * SC
            qr = (bq[2 * ppi: 2 * ppi + 2] + rrb[2 * ppi: 2 * ppi + 2]) * SC
            biasA[:, base + 0 + ppi] = qw.reshape(128)
            biasA[:, base + 2 + ppi] = qr.reshape(128)
            biasA[:, base + 4 + ppi] = \
                qkv_b[l, DM + f0 + ppi * 128: DM + f0 + (ppi + 1) * 128]
        ob = o_w[l][:, fsl] @ qkv_b[l, 2 * DM + f0: 2 * DM + f0 + 256]
        for a in range(NBLK):
            biasA[:, base + 6 + a] = ob[a * 128:(a + 1) * 128]
            biasA[:, base + 26 + a] = ff_b2[l, a * 128:(a + 1) * 128]
            biasA[:, base + 30 + a] = ln1_s[l, a * 128:(a + 1) * 128]
            biasA[:, base + 34 + a] = ln1_b[l, a * 128:(a + 1) * 128]
            biasA[:, base + 38 + a] = ln2_s[l, a * 128:(a + 1) * 128]
            biasA[:, base + 42 + a] = ln2_b[l, a * 128:(a + 1) * 128]
        for m in range(16):
            biasA[:, base + 10 + m] = ff_b1[l, m * 128:(m + 1) * 128]
    d["qkvwT"] = qkvwTa.astype(bf16)
    d["rwT"] = rwTa.astype(bf16)
    d["owT"] = owTa.astype(bf16)
    d["ff1T"] = f1a.astype(bf16)
    d["ff2T"] = f2a.astype(bf16)
    d["bias"] = biasA
    return d


def get_compiled():
    global _COMPILED
    if _COMPILED is None:
        nc = build_program()
        nc.finalize()
        _COMPILED = nc
    return _COMPILED


def run(inputs, trace=False, **kw):
    from concourse import bass_utils
    nc = get_compiled()
    in_maps = [_pack_core_inputs(inputs, c) for c in range(NCORES)]
    res = bass_utils.run_bass_kernel_spmd(
        nc, in_maps, core_ids=list(range(NCORES)), trace=trace, **kw)
    out = np.empty((QLEN, B, DM), np.float32)
    for b_ in range(B):
        for g_ in range(2):
            out[g_ * HTOK:(g_ + 1) * HTOK, b_, :] = \
                res.results[2 * b_ + g_]["yT"].astype(np.float32).T
    return out, res


def kernel(**inputs):
    out, _ = run(inputs, trace=False)
    return out
